# revision 42
# baseline (speedup 1.0000x reference)
"""DeepSeekMoE kernel for 8 TRN2 NeuronCores.

Sharding: load-balanced expert-parallel. Each routed expert's FFN is split
in half along the hidden (H) axis across two cores, and the 4 heaviest
experts (by routed-token count) are paired with the 4 lightest, so every
core carries one heavy half-expert (slot A) and one light half-expert
(slot B) — per-core matmul rows become nearly uniform instead of every
core paying the max expert's padding. Each core also owns a 1/8 H-shard
of the shared expert (tensor-parallel).

The tiny gate (sigmoid + top-2 over E=8) runs on host; tokens are gathered
per expert, padded to per-slot caps (SPMD: one program for all 8 cores),
and shipped pre-transposed so every device-side matmul contracts over the
partition dimension. Each core returns
  yea/yeb: [D, capA/B] half-expert outputs, scaled by the combine weight
  sh:      [T, D]      shared-expert partial (its H-shard, bf16)
Host scatters ye back by token index (the two halves of an expert sum via
the scatter-add) and sums the 8 sh partials — the output gather performs
the MoE combine; no on-device collectives needed.

Device kernel (per core, TensorE-bound):
  warmup (flips the HAM clock gate to 2.4GHz during the initial DMAs)
  B:  hT = gelu(W1half^T x_e)   for slot A then slot B; streamed pair-slabs
  C:  ye = w * (W2half^T hT)    streamed d-slabs, tokens as the moving dim
  D/E interleaved (E lags D by one token-chunk so the 4MB sh output
  streams across the whole shared phase instead of piling up at the tail):
  D:  hsT = gelu(Ws1^T x)       all T tokens through this core's H-shard
  E:  sh = Ws2^T hsT
DMA emission follows usage order: all DGE paths share the 16 SDMA engines,
so later-phase weights stream behind the slabs that gate the PE.

Precision: bf16 operands + fp32 PSUM, with the leading KF8=2 128-chunks of
each routed GEMM's contraction done as one fp8e4m3 DoubleRow matmul (2
rows/cycle): x,W1 pre-scaled by 16/64 clear of e4m3 subnormals (undone by
the activation scale), W2's fp8 rows pre-scaled into the host-folded
combine weight. Error contributions add in quadrature; measured rel err
1.57e-2 vs the 2e-2 gate (bf16-only: 3.4e-3). The shared expert stays
bf16 — its error weight is the largest and its fp8 step would overflow
the budget.
"""

import hashlib
import sys

sys.path.insert(0, "/opt/trn_rl_repo")

import numpy as np
import ml_dtypes

import concourse.bass as bass
import concourse.bacc as bacc
import concourse.mybir as mybir
import concourse.tile as tile
from concourse.bass_utils import run_bass_kernel_spmd

BF16 = ml_dtypes.bfloat16
F32 = np.float32

T, D, E, TOP_K, H = 2048, 1024, 8, 2, 4096
H2 = H // 2          # half-expert hidden
HS = H // 8          # shared-expert hidden shard per core
KD = D // 128        # 8  k-chunks over D
KF8 = 2              # leading D-chunks of routed GEMM1 done in fp8 DoubleRow
KB = KD - KF8        # remaining bf16 D-chunks of routed GEMM1
KH2 = H2 // 128      # 16 k-chunks over a half-expert
KHS = HS // 128      # 4  k-chunks over the shared shard
N_CORES = 8
SX, SW = 16.0, 64.0  # fp8 pre-scales for x and W1 (undone by activation scale)

_DT = mybir.dt.bfloat16
_cache: dict = {}
_wcache: dict = {}


def _tchunks(cap):
    """Split cap into equal (16-multiple) chunks of <=512 columns: a 544-col
    slot becomes (272, 272), not (512, 32) — tiny tail matmuls waste PE."""
    n = -(-cap // 512)
    base = (cap // n) // 16 * 16
    out, s = [], 0
    for i in range(n):
        tsz = cap - s if i == n - 1 else base
        out.append((s, tsz))
        s += tsz
    return out


def _build(caps):
    """Build + finalize the SPMD device program for slot caps (capA, capB)."""
    nc = bacc.Bacc("TRN2", target_bir_lowering=False, debug=False)

    xe_d, w1_d, w2_d, b1_d, wr_d, ye_d = {}, {}, {}, {}, {}, {}
    xef8_d, w1f8_d, w2f8_d = {}, {}, {}
    _F8 = mybir.dt.float8e4
    for s, cap in zip("ab", caps):
        assert cap % 32 == 0
        xe_d[s] = nc.dram_tensor(f"xe{s}", [128, KB, cap], _DT, kind="ExternalInput")
        xef8_d[s] = nc.dram_tensor(f"xef8{s}", [128, KF8, cap], _F8, kind="ExternalInput")
        w1_d[s] = nc.dram_tensor(f"w1{s}", [KH2 // 2, 128, KB, 256], _DT, kind="ExternalInput")
        w1f8_d[s] = nc.dram_tensor(f"w1f8{s}", [128, KF8, H2], _F8, kind="ExternalInput")
        w2_d[s] = nc.dram_tensor(f"w2{s}", [8, 128, KH2 - KF8, 128], _DT, kind="ExternalInput")
        w2f8_d[s] = nc.dram_tensor(f"w2f8{s}", [128, KF8, D], _F8, kind="ExternalInput")
        b1_d[s] = nc.dram_tensor(f"b1{s}", [128, KH2], mybir.dt.float32, kind="ExternalInput")
        wr_d[s] = nc.dram_tensor(f"wr{s}", [128, cap], mybir.dt.float32, kind="ExternalInput")
        ye_d[s] = nc.dram_tensor(f"ye{s}", [D, cap], _DT, kind="ExternalOutput")
    xt_d = nc.dram_tensor("xt", [4, 128, KD, 512], _DT, kind="ExternalInput")
    ws1_d = nc.dram_tensor("ws1", [128, KD, HS], _DT, kind="ExternalInput")
    ws2_d = nc.dram_tensor("ws2", [128, KHS, D], _DT, kind="ExternalInput")
    bs1_d = nc.dram_tensor("bs1c", [128, KHS], mybir.dt.float32, kind="ExternalInput")
    sh_d = nc.dram_tensor("sh", [T, D], _DT, kind="ExternalOutput")

    gelu = mybir.ActivationFunctionType.Gelu

    with tile.TileContext(nc) as tc:
        with (
            tc.tile_pool(name="resident", bufs=1) as rpool,
            tc.tile_pool(name="w1s", bufs=4) as w1pool,
            tc.tile_pool(name="w2s", bufs=4) as w2pool,
            tc.tile_pool(name="xts", bufs=3) as xtpool,
            tc.tile_pool(name="psum", bufs=7, space="PSUM") as pspool,
            tc.tile_pool(name="wpsum", bufs=1, space="PSUM") as wpspool,
            tc.tile_pool(name="outs", bufs=6) as opool,
        ):
            # ---- phase-B-critical loads, in first-consumption order:
            # xe slot-a gates the whole phase, so it goes out first.
            w1s0 = w1pool.tile([128, KB, 256], _DT)
            xe_sb, xef8_sb, wr_sb, b1_sb, hT, w1f8_sb = {}, {}, {}, {}, {}, {}
            xe_sb["a"] = rpool.tile([128, KB, caps[0]], _DT, name="xea", tag="xea")
            xef8_sb["a"] = rpool.tile([128, KF8, caps[0]], _F8, name="xef8a", tag="xef8a")
            w1f8_sb["a"] = rpool.tile([128, KF8, H2], _F8, name="w1f8a", tag="w1f8a")
            # whole-tensor loads in strict first-consumption order: fewer
            # sync-queue trigger slots beats finer-grained arrival here
            # only the slices gating the first (h0,tc0) tile go first: xef8,
            # w1f8's h0-1 slice, xe's chunk-0 columns, w1's col-0 half
            ca0 = _tchunks(caps[0])[0][1]
            nc.sync.dma_start(xef8_sb["a"][:], xef8_d["a"][:])
            nc.sync.dma_start(w1f8_sb["a"][:, :, 0:256], w1f8_d["a"][:, :, 0:256])
            nc.sync.dma_start(xe_sb["a"][:, :, 0:ca0], xe_d["a"][:, :, 0:ca0])
            nc.sync.dma_start(w1s0[:, :, 0:128], w1_d["a"][0, :, :, 0:128])
            nc.sync.dma_start(w1s0[:, :, 128:256], w1_d["a"][0, :, :, 128:256])
            if ca0 < caps[0]:
                nc.sync.dma_start(xe_sb["a"][:, :, ca0:], xe_d["a"][:, :, ca0:])
            nc.sync.dma_start(w1f8_sb["a"][:, :, 256:], w1f8_d["a"][:, :, 256:])
            b1_sb["a"] = rpool.tile([128, KH2], mybir.dt.float32, name="b1a", tag="b1a")
            nc.sync.dma_start(b1_sb["a"][:], b1_d["a"][:])

            # ---- PE warmup: dummy matmuls while the loads above are in
            # flight, so the HAM clock gate is at 2.4GHz for the real work.
            # ~12 x 512-col at ramp clock covers the ~4.7us until xe lands.
            scratch = rpool.tile([128, 512], _DT)
            nc.vector.memset(scratch[:], 0.0)
            wps = wpspool.tile([128, 512], mybir.dt.float32)
            for _ in range(12):
                nc.tensor.matmul(wps[:], scratch[:, 0:128], scratch[:], start=True, stop=True)

            hT["a"] = rpool.tile([128, KH2, caps[0]], _DT, name="hTa", tag="hTa")
            hT["b"] = rpool.tile([128, KH2, caps[1]], _DT, name="hTb", tag="hTb")
            hTf8 = {
                "a": rpool.tile([128, KF8, caps[0]], _F8, name="hTf8a", tag="hTf8a"),
                "b": rpool.tile([128, KF8, caps[1]], _F8, name="hTf8b", tag="hTf8b"),
            }
            hsT = rpool.tile([128, KHS, T], _DT)

            # ---- phase B: routed GEMM1 per slot ----
            for si, s in enumerate("ab"):
                cap = caps[si]
                if s == "b":
                    xe_sb["b"] = rpool.tile([128, KB, cap], _DT, name="xeb", tag="xeb")
                    xef8_sb["b"] = rpool.tile([128, KF8, cap], _F8, name="xef8b", tag="xef8b")
                    w1f8_sb["b"] = rpool.tile([128, KF8, H2], _F8, name="w1f8b", tag="w1f8b")
                    nc.sync.dma_start(xef8_sb["b"][:], xef8_d["b"][:])
                    nc.sync.dma_start(xe_sb["b"][:], xe_d["b"][:])
                    nc.sync.dma_start(w1f8_sb["b"][:], w1f8_d["b"][:])
                    b1_sb["b"] = rpool.tile([128, KH2], mybir.dt.float32, name="b1b", tag="b1b")
                    nc.sync.dma_start(b1_sb["b"][:], b1_d["b"][:])
                for hp in range(KH2 // 2):
                    if s == "a" and hp == 0:
                        w1s = w1s0
                        # chunk-outer: matches DMA arrival (w1 col1 lands
                        # before xe's chunk-1 columns)
                        order = [(hh, tch) for tch in _tchunks(cap) for hh in range(2)]
                    else:
                        w1s = w1pool.tile([128, KB, 256], _DT)
                        nc.sync.dma_start(w1s[:], w1_d[s][hp])
                        order = [(hh, tch) for hh in range(2) for tch in _tchunks(cap)]
                    for hh, (t0, tsz) in order:
                        h = 2 * hp + hh
                        if True:
                            ps = pspool.tile([128, 512], mybir.dt.float32)
                            # k-chunks 0..KF8-1 in one fp8 DoubleRow matmul
                            nc.tensor.matmul(
                                ps[:, :tsz],
                                w1f8_sb[s][:, :, h * 128:(h + 1) * 128],
                                xef8_sb[s][:, :, t0:t0 + tsz],
                                start=True,
                                stop=False,
                                perf_mode=mybir.MatmulPerfMode.DoubleRow,
                            )
                            for k in range(KB):
                                nc.tensor.matmul(
                                    ps[:, :tsz],
                                    w1s[:, k, hh * 128:hh * 128 + 128],
                                    xe_sb[s][:, k, t0:t0 + tsz],
                                    start=False,
                                    stop=(k == KB - 1),
                                )
                            nc.scalar.activation(
                                hT[s][:, h, t0:t0 + tsz], ps[:, :tsz], gelu,
                                bias=b1_sb[s][:, h:h + 1], scale=1.0 / (SX * SW),
                            )
                            if h < KF8:
                                # fp8 copy of the leading GEMM2 k-chunks,
                                # straight from PSUM (single rounding)
                                nc.scalar.activation(
                                    hTf8[s][:, h, t0:t0 + tsz], ps[:, :tsz], gelu,
                                    bias=b1_sb[s][:, h:h + 1], scale=1.0 / (SX * SW),
                                )

            # ---- phase C: routed GEMM2 (tokens moving) + weight scale ----
            ws1_sb = rpool.tile([128, KD, HS], _DT)
            ws2_sb = rpool.tile([128, KHS, D], _DT)
            bs1_sb = rpool.tile([128, KHS], mybir.dt.float32)
            w2f8_sb = {}
            for si, s in enumerate("ab"):
                cap = caps[si]
                wr_sb[s] = rpool.tile([128, cap], mybir.dt.float32, name=f"wr{s}", tag=f"wr{s}")
                nc.sync.dma_start(wr_sb[s][:], wr_d[s][:])
                w2f8_sb[s] = rpool.tile([128, KF8, D], _F8, name=f"w2f8{s}", tag=f"w2f8{s}")
                nc.sync.dma_start(w2f8_sb[s][:], w2f8_d[s][:])
                for d in range(8):
                    w2s = w2pool.tile([128, KH2 - KF8, 128], _DT)
                    nc.sync.dma_start(w2s[:], w2_d[s][d])
                    # shared-expert loads trickle behind the early slabs
                    if s == "a" and d == 0:
                        nc.sync.dma_start(ws1_sb[:], ws1_d[:])
                    elif s == "a" and d == 1:
                        nc.sync.dma_start(ws2_sb[:], ws2_d[:])
                    elif s == "a" and d == 2:
                        nc.sync.dma_start(bs1_sb[:], bs1_d[:])
                    for (t0, tsz) in _tchunks(cap):
                        ps = pspool.tile([128, 512], mybir.dt.float32)
                        nc.tensor.matmul(
                            ps[:, :tsz],
                            w2f8_sb[s][:, :, d * 128:(d + 1) * 128],
                            hTf8[s][:, :, t0:t0 + tsz],
                            start=True,
                            stop=False,
                            perf_mode=mybir.MatmulPerfMode.DoubleRow,
                        )
                        for k in range(KH2 - KF8):
                            nc.tensor.matmul(
                                ps[:, :tsz],
                                w2s[:, k, :],
                                hT[s][:, KF8 + k, t0:t0 + tsz],
                                start=False,
                                stop=(k == KH2 - KF8 - 1),
                            )
                        eo = opool.tile([128, 512], _DT, tag="eo")
                        nc.vector.tensor_mul(
                            eo[:, :tsz], ps[:, :tsz], wr_sb[s][:, t0:t0 + tsz]
                        )
                        nc.sync.dma_start(
                            ye_d[s][d * 128:(d + 1) * 128, t0:t0 + tsz], eo[:, :tsz]
                        )

            # ---- phases D/E: shared expert, E lagging D by one token-chunk
            # so the 4MB sh output streams out across the whole shared phase
            # instead of piling up in the DMA queues at the kernel tail.
            def phase_d(tcn):
                xts = xtpool.tile([128, KD, 512], _DT)
                nc.sync.dma_start(xts[:], xt_d[tcn])
                for hs in range(KHS):
                    ps = pspool.tile([128, 512], mybir.dt.float32)
                    for k in range(KD):
                        nc.tensor.matmul(
                            ps[:],
                            ws1_sb[:, k, hs * 128:(hs + 1) * 128],
                            xts[:, k, :],
                            start=(k == 0),
                            stop=(k == KD - 1),
                        )
                    nc.scalar.activation(
                        hsT[:, hs, tcn * 512:(tcn + 1) * 512], ps[:], gelu,
                        bias=bs1_sb[:, hs:hs + 1],
                    )

            def phase_e(tcn):
                for tt in range(4):
                    t = tcn * 4 + tt
                    for dh in range(2):
                        ps = pspool.tile([128, 512], mybir.dt.float32)
                        for k in range(KHS):
                            nc.tensor.matmul(
                                ps[:],
                                hsT[:, k, t * 128:(t + 1) * 128],
                                ws2_sb[:, k, dh * 512:(dh + 1) * 512],
                                start=(k == 0),
                                stop=(k == KHS - 1),
                            )
                        so = opool.tile([128, 512], _DT, tag="so")
                        if tcn == 3 and tt == 3 and dh == 1:
                            # last tile: halve the cast across Vector+Scalar,
                            # each engine triggering its own DMA — skips the
                            # sync-queue trigger serialization at the tail
                            nc.vector.tensor_copy(so[:, 0:256], ps[:, 0:256])
                            nc.sync.dma_start(
                                sh_d[t * 128:(t + 1) * 128, dh * 512:dh * 512 + 256],
                                so[:, 0:256],
                            )
                            nc.scalar.activation(
                                so[:, 256:512], ps[:, 256:512],
                                mybir.ActivationFunctionType.Copy,
                            )
                            nc.sync.dma_start(
                                sh_d[t * 128:(t + 1) * 128, dh * 512 + 256:(dh + 1) * 512],
                                so[:, 256:512],
                            )
                        else:
                            nc.vector.tensor_copy(so[:], ps[:])
                            nc.sync.dma_start(
                                sh_d[t * 128:(t + 1) * 128, dh * 512:(dh + 1) * 512], so[:]
                            )

            phase_d(0)
            phase_d(1)
            phase_e(0)
            phase_d(2)
            phase_e(1)
            phase_d(3)
            phase_e(2)
            phase_e(3)

    nc.finalize()
    return nc


def _routing(xf, Wg, bg, bias):
    """Host gate: fp64 for a stable top-2 ranking (matches fp32 reference
    ordering except for ~1e-7-wide ties, which don't occur at these margins)."""
    logits = xf.astype(np.float64) @ Wg.T.astype(np.float64) + bg + bias
    scores = (1.0 / (1.0 + np.exp(-logits))).astype(np.float32)
    # stable sort => ties break toward the lower expert index, like lax.top_k
    top_idx = np.argsort(-scores, axis=1, kind="stable")[:, :TOP_K]
    top_w = np.take_along_axis(scores, top_idx, axis=1)
    return top_idx, top_w


def _round32(n):
    return max(64, -(-n // 32) * 32)


def kernel(x, Wg, bg, bias, W1, b1, W2, b2, Ws1, bs1, Ws2, bs2):
    x = np.asarray(x, F32)
    Wg, bg, bias = np.asarray(Wg, F32), np.asarray(bg, F32), np.asarray(bias, F32)
    W1, b1 = np.asarray(W1, F32), np.asarray(b1, F32)
    W2, b2 = np.asarray(W2, F32), np.asarray(b2, F32)
    Ws1, bs1 = np.asarray(Ws1, F32), np.asarray(bs1, F32)
    Ws2, bs2 = np.asarray(Ws2, F32), np.asarray(bs2, F32)

    xf = x.reshape(-1, D)
    top_idx, top_w = _routing(xf, Wg, bg, bias)

    sels, ws = [], []
    for e in range(E):
        pick = (top_idx == e)
        sel = np.where(pick.any(axis=1))[0]
        w = np.where(pick[sel, 0], top_w[sel, 0], top_w[sel, 1]).astype(F32)
        sels.append(sel)
        ws.append(w)
    counts = np.array([len(s) for s in sels])
    order = np.argsort(-counts, kind="stable")
    heavy, light = order[:4], order[4:]
    caps = (_round32(counts[heavy].max()), _round32(counts[light].max()))

    if caps not in _cache:
        _cache[caps] = _build(caps)
    nc = _cache[caps]

    x_bf = xf.astype(BF16)
    # xt: [4, 128, KD, 512]  (token-chunk major, partition-major inside)
    xt = np.ascontiguousarray(
        x_bf.T.reshape(KD, 128, 4, 512).transpose(2, 1, 0, 3)
    )

    # Half-expert weight re-layouts are input-independent; cache across calls
    # (keyed by content hash, so a reused buffer can't serve stale layouts).
    hsh = hashlib.blake2b(digest_size=16)
    for a in (W1, W2, Ws1, Ws2, b1, bs1):
        hsh.update(np.ascontiguousarray(a).data)
    wkey = hsh.hexdigest()
    wmaps = _wcache.get(wkey)
    if wmaps is None:
        wmaps = {"half": {}, "core": []}
        F8 = ml_dtypes.float8_e4m3fn
        DF8 = KF8 * 128
        for e in range(E):
            for hf in range(2):
                r0 = hf * H2
                wmaps["half"][(e, hf)] = {
                    # bf16 part: W1 half rows, D-cols DF8.. -> [8, 128, KB, 256]
                    # pre-scaled by SW so it matches the fp8 chunks' scale
                    "w1": np.ascontiguousarray(
                        (W1[e][r0:r0 + H2, DF8:] * SW).T
                        .reshape(KB, 128, KH2 // 2, 256)
                        .transpose(2, 1, 0, 3).astype(BF16)
                    ),
                    # fp8 part: D-cols 0..DF8 -> DoubleRow layout [128, KF8, H2]
                    "w1f8": np.ascontiguousarray(
                        (W1[e][r0:r0 + H2, 0:DF8] * SW).astype(F8).T
                        .reshape(KF8, 128, H2).transpose(1, 0, 2)
                    ),
                    # W2 half cols, h-rows DF8.. -> [8, 128, KH2-KF8, 128]
                    "w2": np.ascontiguousarray(
                        (W2[e][:, r0 + DF8:r0 + H2] * SW).T
                        .reshape(KH2 - KF8, 128, 8, 128)
                        .transpose(2, 1, 0, 3).astype(BF16)
                    ),
                    # fp8 part: h-rows r0..r0+DF8 -> [128, KF8, D]
                    "w2f8": np.ascontiguousarray(
                        (W2[e][:, r0:r0 + DF8] * SW).astype(F8).T
                        .reshape(KF8, 128, D).transpose(1, 0, 2)
                    ),
                    "b1": np.ascontiguousarray(b1[e][r0:r0 + H2].reshape(KH2, 128).T),
                }
        for c in range(N_CORES):
            hs0 = c * HS
            wmaps["core"].append({
                "ws1": np.ascontiguousarray(
                    Ws1[hs0:hs0 + HS].T.reshape(KD, 128, HS)
                    .transpose(1, 0, 2).astype(BF16)
                ),
                "ws2": np.ascontiguousarray(
                    Ws2[:, hs0:hs0 + HS].T.reshape(KHS, 128, D)
                    .transpose(1, 0, 2).astype(BF16)
                ),
                "bs1c": np.ascontiguousarray(bs1[hs0:hs0 + HS].reshape(KHS, 128).T),
            })
        _wcache.clear()
        _wcache[wkey] = wmaps

    # per-expert gathered tokens + combine weights at the slot cap.
    # Routed GEMM1 operands are pre-scaled by SX (undone on device) so the
    # fp8 chunks stay clear of e4m3's subnormal range.
    F8 = ml_dtypes.float8_e4m3fn
    DF8 = KF8 * 128
    x16 = xf * np.float32(SX)
    x16_bf = x16[:, DF8:].astype(BF16)
    x16_f8 = x16[:, 0:DF8].astype(F8)

    def gathered(e, cap):
        sel, w = sels[e], ws[e]
        xeb = np.zeros((cap, D - DF8), BF16)
        xeb[: len(sel)] = x16_bf[sel]
        xe_t = np.ascontiguousarray(xeb.T.reshape(KB, 128, cap).transpose(1, 0, 2))
        xe8 = np.zeros((cap, DF8), F8)
        xe8[: len(sel)] = x16_f8[sel]
        xe8_t = np.ascontiguousarray(xe8.T.reshape(KF8, 128, cap).transpose(1, 0, 2))
        wpad = np.zeros(cap, F32)
        wpad[: len(w)] = w / np.float32(SW)  # undoes the W2 pre-scale
        wr = np.ascontiguousarray(np.broadcast_to(wpad, (128, cap)))
        return xe_t, xe8_t, wr

    gcache = {}
    in_maps = []
    for c in range(N_CORES):
        m = {"xt": xt, **wmaps["core"][c]}
        for s, grp, cap in (("a", heavy, caps[0]), ("b", light, caps[1])):
            e, hf = int(grp[c // 2]), c % 2
            if e not in gcache:
                gcache[e] = gathered(e, cap)
            m[f"xe{s}"], m[f"xef8{s}"], m[f"wr{s}"] = gcache[e]
            half = wmaps["half"][(e, hf)]
            m[f"w1{s}"], m[f"w2{s}"], m[f"b1{s}"] = half["w1"], half["w2"], half["b1"]
            m[f"w1f8{s}"], m[f"w2f8{s}"] = half["w1f8"], half["w2f8"]
        in_maps.append(m)

    res = run_bass_kernel_spmd(nc, in_maps, core_ids=list(range(N_CORES)))

    out = np.zeros((T, D), F32)
    for c in range(N_CORES):
        out += res.results[c]["sh"].astype(F32)
        for s, grp in (("a", heavy), ("b", light)):
            e = int(grp[c // 2])
            sel = sels[e]
            out[sel] += res.results[c][f"ye{s}"][:, : len(sel)].T.astype(F32)
    # biases handled host-side: per-token weighted b2, plus bs2
    wdense = np.zeros((T, E), F32)
    np.put_along_axis(wdense, top_idx, top_w, axis=1)
    out += wdense @ b2
    out += bs2
    return out.reshape(x.shape)

